# revision 1
# baseline (speedup 1.0000x reference)
"""Longformer sliding-chunk attention (B=2, S=4096, E=1024, H=16, W=256) on 8 trn2 cores.

Sharding: tensor-parallel over heads — core c owns heads {2c, 2c+1}. Each core:
  - projects q/k/v for its 128 output features (2 heads x 64) over the full
    [8192, 1024] hidden states, directly in transposed [d, s] layout
  - computes chunked attention fully transposed: scoresT = K @ Q^T per
    128-key-block, exp on ACT (no max subtraction: scores are O(1) for this
    problem), probsT @ V via PE with an appended ones-column that yields the
    softmax denominators for free
  - ships unnormalized numerator^T [128, 8192] + denominators [2, 8192]
Host adds the boundary-mask pad mass to denominators and normalizes.

All matmuls run in float32r (full-rate fp32 mode, ~1e-4 rounding).
"""
import numpy as np

import concourse.bass as bass
import concourse.mybir as mybir
import concourse.tile as tile
from concourse import bacc
from concourse.bass_utils import run_bass_kernel_spmd
from concourse.masks import make_identity

F32 = mybir.dt.float32
F32R = mybir.dt.float32r
AFT = mybir.ActivationFunctionType

B, S, E = 2, 4096, 1024
H, W, D = 16, 256, 64
BS = B * S           # 8192
NT = 16              # 512-wide seq tiles over BS for projections
KT = 8               # contraction tiles of 128 over E
NCHUNK = S // W      # 16 chunks per batch
NKB = S // 128       # 32 key blocks of 128 per batch

_NC_CACHE = None


def _build():
    nc = bacc.Bacc("TRN2", target_bir_lowering=False, debug=False, num_devices=8)

    hsT = nc.dram_tensor("hsT", [E, BS], F32R, kind="ExternalInput").ap()
    w_ap = {}
    b_ap = {}
    for nm in ("q", "k", "v"):
        w_ap[nm] = nc.dram_tensor(f"w{nm}T", [E, 128], F32R, kind="ExternalInput").ap()
        b_ap[nm] = nc.dram_tensor(f"b{nm}", [128, 1], F32, kind="ExternalInput").ap()
    ones2 = nc.dram_tensor("ones2", [128, 24], F32R, kind="ExternalInput").ap()
    outT = nc.dram_tensor("outT", [130, BS], F32, kind="ExternalOutput").ap()

    with tile.TileContext(nc) as tc:
        with (
            tc.tile_pool(name="singles", bufs=1) as singles,
            tc.tile_pool(name="big", bufs=1) as big,
            tc.tile_pool(name="hst", bufs=4) as hpool,
            tc.tile_pool(name="probs", bufs=4) as probs_pool,
            tc.tile_pool(name="vones", bufs=12) as vpool,
            tc.tile_pool(name="stage", bufs=4) as stage_pool,
            tc.tile_pool(name="psmm", bufs=4, space="PSUM") as ps_mm,
            tc.tile_pool(name="pspv", bufs=2, space="PSUM") as ps_pv,
            tc.tile_pool(name="psvt", bufs=2, space="PSUM") as ps_vt,
        ):
            ident = singles.tile([128, 128], F32)
            make_identity(nc, ident)

            w_sb = {}
            b_sb = {}
            for nm in ("q", "k", "v"):
                wt = singles.tile([128, KT, 128], F32R, tag=f"w{nm}")
                nc.sync.dma_start(
                    out=wt, in_=w_ap[nm].rearrange("(kt p) m -> p kt m", p=128)
                )
                w_sb[nm] = wt
                bt = singles.tile([128, 1], F32, tag=f"b{nm}")
                nc.sync.dma_start(out=bt, in_=b_ap[nm])
                b_sb[nm] = bt

            QT = big.tile([128, BS], F32R, tag="qt")
            vring = big.tile([128, 12, 130], F32R, tag="vring")
            nc.sync.dma_start(
                out=vring.rearrange("p s (x o) -> p s x o", x=2)[:, :, :, 64:65],
                in_=ones2.rearrange("p (s x o) -> p s x o", s=12, x=2, o=1),
            )
            KTt = big.tile([128, BS], F32R, tag="kt")
            VT = big.tile([128, BS], F32R, tag="vt")

            # ---- Phase 1: projections, output in [feature, seq] layout ----
            hsT_r = hsT.rearrange("(kt p) s -> p kt s", p=128)
            for n in range(NT):
                sl = slice(n * 512, (n + 1) * 512)
                hst0 = hpool.tile([128, 4, 512], F32R, tag="hst")
                hst1 = hpool.tile([128, 4, 512], F32R, tag="hst")
                nc.sync.dma_start(out=hst0, in_=hsT_r[:, 0:4, sl])
                nc.sync.dma_start(out=hst1, in_=hsT_r[:, 4:8, sl])
                halves = (hst0, hst1)
                for nm, dest, scale in (
                    ("q", QT, 1.0 / np.sqrt(D)),
                    ("k", KTt, 1.0),
                    ("v", VT, 1.0),
                ):
                    psp = ps_mm.tile([128, 512], F32, tag="mm")
                    for k in range(KT):
                        nc.tensor.matmul(
                            psp,
                            lhsT=w_sb[nm][:, k, :],
                            rhs=halves[k // 4][:, k % 4, :],
                            start=(k == 0),
                            stop=(k == KT - 1),
                        )
                    nc.scalar.activation(
                        dest[:, sl], psp, AFT.Identity, bias=b_sb[nm], scale=scale
                    )

            # ---- Phase 2: chunked attention, fully transposed ----
            vones = {}
            for b in range(B):
                base = b * S
                for c in range(NCHUNK):
                    lo = max(0, 2 * c - 2)
                    hi = min(NKB, 2 * c + 4)
                    n_kb = hi - lo

                    # V^T -> [keys, d] ring slots (+persistent ones col)
                    for kb in range(lo, hi):
                        if (b, kb) in vones:
                            continue
                        vt_ps = ps_vt.tile([128, 128], F32, tag="vt")
                        nc.tensor.transpose(
                            vt_ps,
                            VT[:, base + kb * 128 : base + (kb + 1) * 128].bitcast(F32),
                            ident,
                        )
                        slot = (2 * NKB * b + kb) % 12
                        nc.vector.tensor_copy(
                            vring[:, slot, :].rearrange("p (h x) -> p h x", h=2)[
                                :, :, 0:64
                            ],
                            vt_ps.rearrange("p (h x) -> p h x", h=2),
                        )
                        vones[(b, kb)] = slot

                    q_sl = slice(base + c * W, base + (c + 1) * W)
                    pr = {
                        h: probs_pool.tile(
                            [128, 6, 256], F32R, tag="probs", name=f"pr{h}_{b}_{c}"
                        )
                        for h in (0, 1)
                    }
                    for ip in range(n_kb // 2):
                        sps = {
                            h: ps_mm.tile(
                                [128, 2, 256], F32, tag="mm", name=f"s{h}_{b}_{c}_{ip}"
                            )
                            for h in (0, 1)
                        }
                        for j in (0, 1):
                            kb = lo + 2 * ip + j
                            k_sl = slice(base + kb * 128, base + (kb + 1) * 128)
                            for h in (0, 1):
                                d_sl = slice(h * 64, (h + 1) * 64)
                                nc.tensor.matmul(
                                    sps[h][:, j, :],
                                    lhsT=KTt[d_sl, k_sl],
                                    rhs=QT[d_sl, q_sl],
                                    start=True,
                                    stop=True,
                                )
                        for h in (0, 1):
                            nc.scalar.activation(
                                pr[h][:, 2 * ip : 2 * ip + 2, :], sps[h], AFT.Exp
                            )

                    o_sl_pre = slice(base + c * W, base + (c + 1) * W)
                    stage = stage_pool.tile([128, 256], F32, tag="stage")
                    for h in (0, 1):
                        po = ps_pv.tile([65, 256], F32, tag="pv")
                        for i in range(n_kb):
                            kb = lo + i
                            nc.tensor.matmul(
                                po,
                                lhsT=vring[:, vones[(b, kb)], h * 65 : (h + 1) * 65],
                                rhs=pr[h][:, i, :],
                                start=(i == 0),
                                stop=(i == n_kb - 1),
                            )
                        nc.vector.tensor_copy(stage[h * 64 : (h + 1) * 64, :], po[0:64, :])
                        dst_h = stage_pool.tile(
                            [1, 256], F32, tag=f"dstage{h}", name=f"dst{h}_{b}_{c}"
                        )
                        nc.vector.tensor_copy(dst_h, po[64:65, :])
                        nc.sync.dma_start(
                            out=outT[128 + h : 129 + h, o_sl_pre], in_=dst_h
                        )

                    nc.sync.dma_start(out=outT[0:128, o_sl_pre], in_=stage)

    nc.compile()
    return nc


def get_nc():
    global _NC_CACHE
    if _NC_CACHE is None:
        _NC_CACHE = _build()
    return _NC_CACHE


def make_in_maps(hidden_states, Wq, bq, Wk, bk, Wv, bv):
    hsT = np.ascontiguousarray(
        hidden_states.reshape(BS, E).T.astype(np.float32, copy=False)
    )
    ones2 = np.ones((128, 24), np.float32)
    in_maps = []
    for c in range(8):
        fsl = slice(c * 128, (c + 1) * 128)
        in_maps.append(
            {
                "hsT": hsT,
                "wqT": np.ascontiguousarray(Wq[fsl].T.astype(np.float32, copy=False)),
                "wkT": np.ascontiguousarray(Wk[fsl].T.astype(np.float32, copy=False)),
                "wvT": np.ascontiguousarray(Wv[fsl].T.astype(np.float32, copy=False)),
                "bq": np.ascontiguousarray(bq[fsl].reshape(128, 1) / np.sqrt(D)),
                "bk": np.ascontiguousarray(bk[fsl].reshape(128, 1)),
                "bv": np.ascontiguousarray(bv[fsl].reshape(128, 1)),
                "ones2": ones2,
            }
        )
    return in_maps


def assemble(results):
    """results: list of 8 per-core dicts with 'outT' [130, BS] -> full [B,S,E]."""
    # boundary pad mass: chunk 0 row ii has ii unmasked zero-score pad keys,
    # chunk 15 row ii has 255-ii
    pad = np.zeros(S, np.float32)
    pad[:W] = np.arange(W, dtype=np.float32)
    pad[S - W :] = (W - 1) - np.arange(W, dtype=np.float32)

    out = np.empty((B, S, E), np.float32)
    for c in range(8):
        oT = results[c]["outT"]  # [130, BS]
        num = oT[0:128].T.reshape(B, S, 2, 64)  # b, s, head_local, d
        den = oT[128:130].T.reshape(B, S, 2)  # b, s, head_local
        den = den + pad[None, :, None]
        out[:, :, c * 128 : (c + 1) * 128] = (num / den[..., None]).reshape(B, S, 128)
    return out


def kernel(hidden_states, Wq, bq, Wk, bk, Wv, bv):
    nc = get_nc()
    in_maps = make_in_maps(hidden_states, Wq, bq, Wk, bk, Wv, bv)
    res = run_bass_kernel_spmd(nc, in_maps, list(range(8)))
    return assemble(res.results)



# revision 31
# speedup vs baseline: 337.7365x; 337.7365x over previous
"""Longformer sliding-chunk attention (B=2, S=4096, E=1024, H=16, W=256) on 8 trn2 cores.

Sharding: tensor-parallel over heads — core c owns heads {2c, 2c+1}. Each core:
  - projects q/k for its 128 output features (2 heads x 64) over the full
    [8192, 1024] hidden states in transposed [d, s] layout (lhsT = weights)
  - projects v in [s, d] layout directly (lhsT = hidden-state tile, rhs = Wv^T),
    so no PE transposes are needed; v bias is folded in on the host
  - computes chunked attention fully transposed: scoresT = K @ Q^T per
    128-key-block into a [128, n_kb, 256] PSUM strip, one wide exp ACTIVATE
    per (chunk, head) (no max subtraction: scores are O(1) here), probsT @ V
    via PE with an appended ones-column that yields softmax denominators free
  - ships unnormalized numerator^T + denominators as [130, 8192]:
    rows 0:64 head0 numerator, 64 head0 denom, 65:129 head1 num, 129 h1 denom
Host adds boundary-mask pad mass to denominators, normalizes, and adds bv.

All matmuls run in bfloat16 (fp32 PSUM accumulate); projections+attention are
interleaved per 1024-token tile so exp/bias ACTIVATEs hide under PE matmuls.
"""
import numpy as np
import ml_dtypes

import concourse.bass as bass
import concourse.mybir as mybir
import concourse.tile as tile
from concourse import bacc
from concourse.bass_utils import run_bass_kernel_spmd

F32 = mybir.dt.float32
BF16 = mybir.dt.bfloat16
FP8 = mybir.dt.float8e4
DR = mybir.MatmulPerfMode.DoubleRow
AFT = mybir.ActivationFunctionType

B, S, E = 2, 4096, 1024
H, W, D = 16, 256, 64
BS = B * S           # 8192
NT = 4               # 1024-wide seq tiles per batch
KT = 8               # contraction tiles of 128 over E
NCHUNK = S // W      # 16 chunks per batch
NKB = S // 128       # 32 key blocks of 128 per batch

_NC_CACHE = None
_BV = None           # host-side v bias, folded in during assemble


def _build():
    nc = bacc.Bacc("TRN2", target_bir_lowering=False, debug=False, num_devices=8)

    # host ships hidden states pre-tiled: [128, tile(8), kt(8), seq(1024)] so
    # one projection tile is a single 128-descriptor DMA of 16 KB/descriptor
    hsT = nc.dram_tensor("hsT", [128, (B * S // 1024) * KT * 1024], BF16,
                         kind="ExternalInput").ap()
    w_ap = {}
    for nm in ("q", "k", "v"):
        # host ships weights pre-arranged as [128, KT*128]: row p holds
        # W^T[kt*128+p, :] for kt = 0..7 -> one 2 KB descriptor per partition
        w_ap[nm] = nc.dram_tensor(f"w{nm}T", [128, KT * 128], BF16, kind="ExternalInput").ap()
    b_ap = {
        nm: nc.dram_tensor(f"b{nm}", [128, 1], F32, kind="ExternalInput").ap()
        for nm in ("q", "k")
    }
    outT = nc.dram_tensor("outT", [130, BS], F32, kind="ExternalOutput").ap()

    with tile.TileContext(nc) as tc:
        with (
            tc.tile_pool(name="singles", bufs=1) as singles,
            tc.tile_pool(name="big", bufs=1) as big,
            tc.tile_pool(name="hst", bufs=3) as hpool,
            tc.tile_pool(name="probs", bufs=4) as probs_pool,
            tc.tile_pool(name="stage", bufs=4) as stage_pool,
            tc.tile_pool(name="pssm", bufs=2, space="PSUM") as ps_sm,
            tc.tile_pool(name="pssc", bufs=2, space="PSUM") as ps_sc,
        ):
            hsT_r = hsT.rearrange("p (t kt s) -> p t kt s", kt=KT, s=1024)

            # Tiny first DMA so the DMA queues spin up during the ~5us
            # engine preamble instead of delaying the first weight load.
            warm = singles.tile([128, 16], BF16, tag="warm")
            nc.sync.dma_start(out=warm, in_=w_ap["q"][:, 0:16])

            # DMA issue order tuned for startup: q/k weights first, then the
            # first hidden-state tile, then biases/wv/ones.
            w_sb = {}
            b_sb = {}
            for nm in ("q", "k"):
                wt = singles.tile([128, KT, 128], BF16, tag=f"w{nm}")
                nc.sync.dma_start(
                    out=wt, in_=w_ap[nm].rearrange("p (kt m) -> p kt m", m=128)
                )
                w_sb[nm] = wt

            hst_pre = {}

            def load_hst(b, n, split=False):
                t = b * NT + n
                hst = hpool.tile([128, KT, 1024], BF16, tag="hst")
                # one DMA per contraction slab so matmul k can start as soon
                # as slab k lands
                for k in range(KT):
                    nc.sync.dma_start(
                        out=hst[:, k : k + 1, :], in_=hsT_r[:, t, k : k + 1, :]
                    )
                return hst

            hst_pre[(0, 0)] = load_hst(0, 0, split=True)

            for nm in ("q", "k"):
                bt = singles.tile([128, 1], F32, tag=f"b{nm}")
                nc.sync.dma_start(out=bt, in_=b_ap[nm])
                b_sb[nm] = bt
            wt = singles.tile([128, KT, 128], BF16, tag="wv")
            nc.sync.dma_start(
                out=wt, in_=w_ap["v"].rearrange("p (kt m) -> p kt m", m=128)
            )
            w_sb["v"] = wt

            QT = big.tile([128, BS], BF16, tag="qt")
            KTt = big.tile([128, BS], BF16, tag="kt")
            # v store: per 128-key block, [seq 128, 65*2]: cols 0:64 head0 d,
            # 64 ones, 65:129 head1 d, 129 ones
            vfull = big.tile([128, B * NKB, 130], BF16, tag="vfull")
            nc.vector.memset(
                vfull.rearrange("p s (x o) -> p s x o", x=2)[:, :, :, 64:65], 1.0
            )

            def attn_scores(b, c):
                """Issue score matmuls + exp for chunk c; return PV context."""
                base = b * S
                lo = max(0, 2 * c - 2)
                hi = min(NKB, 2 * c + 4)
                n_kb = hi - lo
                q_sl = slice(base + c * W, base + (c + 1) * W)
                prs = {}
                for h in (0, 1):
                    d_sl = slice(h * 64, (h + 1) * 64)
                    sps = ps_sc.tile([128, 6, 256], F32, tag="sc", name=f"s{h}_{b}_{c}")
                    for i in range(n_kb):
                        kb = lo + i
                        k_sl = slice(base + kb * 128, base + (kb + 1) * 128)
                        nc.tensor.matmul(
                            sps[:, i, :],
                            lhsT=KTt[d_sl, k_sl],
                            rhs=QT[d_sl, q_sl],
                            start=True,
                            stop=True,
                        )
                    pr = probs_pool.tile(
                        [128, 6, 256], BF16, tag="probs", name=f"pr{h}_{b}_{c}"
                    )
                    nc.scalar.activation(
                        pr[:, 0:n_kb, :], sps[:, 0:n_kb, :], AFT.Exp
                    )
                    prs[h] = pr
                return (b, c, lo, n_kb, q_sl, prs)

            def attn_pv(ctx):
                b, c, lo, n_kb, q_sl, prs = ctx
                for h in (0, 1):
                    po = ps_sm.tile([65, 256], F32, tag="sm", name=f"pv{h}_{b}_{c}")
                    for i in range(n_kb):
                        slot = b * NKB + lo + i
                        nc.tensor.matmul(
                            po,
                            lhsT=vfull[:, slot, h * 65 : (h + 1) * 65],
                            rhs=prs[h][:, i, :],
                            start=(i == 0),
                            stop=(i == n_kb - 1),
                        )
                    st = stage_pool.tile([65, 256], F32, tag="stage")
                    nc.vector.tensor_copy(st, po)
                    nc.sync.dma_start(
                        out=outT[h * 65 : (h + 1) * 65, q_sl], in_=st
                    )

            pending = None
            for b in range(B):
                base = b * S
                c_done = 0
                for n in range(NT):
                    hst = hst_pre.pop((b, n), None)
                    if hst is None:
                        hst = load_hst(b, n)
                    # q/k projections: [feat, seq] layout, lhsT = weights
                    for nm, dest, scale, bias in (
                        ("q", QT, 1.0 / np.sqrt(D), b_sb["q"]),
                        ("k", KTt, 1.0, b_sb["k"]),
                    ):
                        for half in (0, 1):
                            hsl = slice(half * 512, (half + 1) * 512)
                            osl = slice(
                                base + n * 1024 + half * 512,
                                base + n * 1024 + (half + 1) * 512,
                            )
                            psp = ps_sm.tile(
                                [128, 512], F32, tag="sm", name=f"p{nm}_{b}_{n}_{half}"
                            )
                            for k in range(KT):
                                nc.tensor.matmul(
                                    psp,
                                    lhsT=w_sb[nm][:, k, :],
                                    rhs=hst[:, k, hsl],
                                    start=(k == 0),
                                    stop=(k == KT - 1),
                                )
                            nc.scalar.activation(
                                dest[:, osl], psp, AFT.Identity, bias=bias, scale=scale
                            )
                    # v projection: [seq, feat] layout, lhsT = hidden tile
                    for j in range(KT):
                        kb = n * 8 + j
                        ssl = slice(j * 128, (j + 1) * 128)
                        psv = ps_sm.tile(
                            [128, 128], F32, tag="sm", name=f"pv_{b}_{n}_{j}"
                        )
                        for k in range(KT):
                            nc.tensor.matmul(
                                psv,
                                lhsT=hst[:, k, ssl],
                                rhs=w_sb["v"][:, k, :],
                                start=(k == 0),
                                stop=(k == KT - 1),
                            )
                        slot = b * NKB + kb
                        nc.vector.tensor_copy(
                            vfull[:, slot, 0:130].rearrange("p (x o) -> p x o", x=2)[
                                :, :, 0:64
                            ],
                            psv.rearrange("p (x o) -> p x o", x=2),
                        )
                    # attention chunks whose window is now fully projected
                    c_hi = NCHUNK if n == NT - 1 else 4 * n + 3
                    for c in range(c_done, c_hi):
                        ctx = attn_scores(b, c)
                        if pending is not None:
                            attn_pv(pending)
                        pending = ctx
                    c_done = c_hi
            if pending is not None:
                attn_pv(pending)

    nc.compile()
    return nc


def get_nc():
    global _NC_CACHE
    if _NC_CACHE is None:
        _NC_CACHE = _build()
    return _NC_CACHE


def _w_prep(w):
    """[128, E] slice of W -> [128 partition, KT*128] with row p holding
    W^T[kt*128+p, :] for kt = 0..KT-1."""
    bf = ml_dtypes.bfloat16
    wt = w.T.reshape(KT, 128, 128)  # [kt, p, m]
    return np.ascontiguousarray(wt.transpose(1, 0, 2).reshape(128, KT * 128)).astype(bf)


def make_in_maps(hidden_states, Wq, bq, Wk, bk, Wv, bv):
    global _BV
    _BV = np.asarray(bv, np.float32)
    bf = ml_dtypes.bfloat16
    # [E, BS] -> [128(p), tile(8), kt(8), 1024] -> [128, 64K], row-contiguous
    hsTf = hidden_states.reshape(BS, E).T  # [E, BS]
    hsT = np.ascontiguousarray(
        hsTf.reshape(KT, 128, B * NT, 1024).transpose(1, 2, 0, 3).reshape(128, -1)
    ).astype(bf)
    in_maps = []
    for c in range(8):
        fsl = slice(c * 128, (c + 1) * 128)
        in_maps.append(
            {
                "hsT": hsT,
                "wqT": _w_prep(Wq[fsl]),
                "wkT": _w_prep(Wk[fsl]),
                "wvT": _w_prep(Wv[fsl]),
                "bq": np.ascontiguousarray(
                    (bq[fsl] / np.sqrt(D)).reshape(128, 1).astype(np.float32)
                ),
                "bk": np.ascontiguousarray(bk[fsl].reshape(128, 1).astype(np.float32)),
            }
        )
    return in_maps


def assemble(results):
    """results: list of 8 per-core dicts with 'outT' [130, BS] -> full [B,S,E]."""
    # boundary pad mass: chunk 0 row ii has ii unmasked zero-score pad keys,
    # chunk 15 row ii has 255-ii
    pad = np.zeros(S, np.float32)
    pad[:W] = np.arange(W, dtype=np.float32)
    pad[S - W :] = (W - 1) - np.arange(W, dtype=np.float32)
    pad_bs = np.tile(pad, B)  # [BS]

    out = np.empty((B, S, E), np.float32)
    for c in range(8):
        oT = np.asarray(results[c]["outT"], np.float32)  # [130, BS]
        for hl in (0, 1):
            num = oT[hl * 65 : hl * 65 + 64].T  # [BS, 64]
            den_dev = oT[hl * 65 + 64]  # [BS]
            den = den_dev + pad_bs
            fsl = slice(c * 128 + hl * 64, c * 128 + (hl + 1) * 64)
            full = (num + den_dev[:, None] * _BV[None, fsl]) / den[:, None]
            out.reshape(BS, E)[:, fsl] = full
    return out


def kernel(hidden_states, Wq, bq, Wk, bk, Wv, bv):
    hidden_states, Wq, bq, Wk, bk, Wv, bv = (
        np.asarray(x, np.float32) for x in (hidden_states, Wq, bq, Wk, bk, Wv, bv)
    )
    nc = get_nc()
    in_maps = make_in_maps(hidden_states, Wq, bq, Wk, bk, Wv, bv)
    res = run_bass_kernel_spmd(nc, in_maps, list(range(8)))
    return assemble(res.results)


# revision 32
# speedup vs baseline: 338.1738x; 1.0013x over previous
"""Longformer sliding-chunk attention (B=2, S=4096, E=1024, H=16, W=256) on 8 trn2 cores.

Sharding: tensor-parallel over heads — core c owns heads {2c, 2c+1}. Each core:
  - projects q/k for its 128 output features (2 heads x 64) over the full
    [8192, 1024] hidden states in transposed [d, s] layout (lhsT = weights)
  - projects v in [s, d] layout directly (lhsT = hidden-state tile, rhs = Wv^T),
    so no PE transposes are needed; v bias is folded in on the host
  - computes chunked attention fully transposed: scoresT = K @ Q^T per
    128-key-block into a [128, n_kb, 256] PSUM strip, one wide exp ACTIVATE
    per (chunk, head) (no max subtraction: scores are O(1) here), probsT @ V
    via PE with an appended ones-column that yields softmax denominators free
  - ships unnormalized numerator^T + denominators as [130, 8192]:
    rows 0:64 head0 numerator, 64 head0 denom, 65:129 head1 num, 129 h1 denom
Host adds boundary-mask pad mass to denominators, normalizes, and adds bv.

All matmuls run in bfloat16 (fp32 PSUM accumulate); projections+attention are
interleaved per 1024-token tile so exp/bias ACTIVATEs hide under PE matmuls.
"""
import numpy as np
import ml_dtypes

import concourse.bass as bass
import concourse.mybir as mybir
import concourse.tile as tile
from concourse import bacc
from concourse.bass_utils import run_bass_kernel_spmd

F32 = mybir.dt.float32
BF16 = mybir.dt.bfloat16
AFT = mybir.ActivationFunctionType

B, S, E = 2, 4096, 1024
H, W, D = 16, 256, 64
BS = B * S           # 8192
NT = 4               # 1024-wide seq tiles per batch
KT = 8               # contraction tiles of 128 over E
NCHUNK = S // W      # 16 chunks per batch
NKB = S // 128       # 32 key blocks of 128 per batch

_NC_CACHE = None
_BV = None           # host-side v bias, folded in during assemble


def _build():
    nc = bacc.Bacc("TRN2", target_bir_lowering=False, debug=False, num_devices=8)

    # host ships hidden states pre-tiled: [128, tile(8), kt(8), seq(1024)] so
    # one projection tile is a single 128-descriptor DMA of 16 KB/descriptor
    hsT = nc.dram_tensor("hsT", [128, (B * S // 1024) * KT * 1024], BF16,
                         kind="ExternalInput").ap()
    w_ap = {}
    for nm in ("q", "k", "v"):
        # host ships weights pre-arranged as [128, KT*128]: row p holds
        # W^T[kt*128+p, :] for kt = 0..7 -> one 2 KB descriptor per partition
        w_ap[nm] = nc.dram_tensor(f"w{nm}T", [128, KT * 128], BF16, kind="ExternalInput").ap()
    b_ap = {
        nm: nc.dram_tensor(f"b{nm}", [128, 1], F32, kind="ExternalInput").ap()
        for nm in ("q", "k")
    }
    outT = nc.dram_tensor("outT", [130, BS], F32, kind="ExternalOutput").ap()

    with tile.TileContext(nc) as tc:
        with (
            tc.tile_pool(name="singles", bufs=1) as singles,
            tc.tile_pool(name="big", bufs=1) as big,
            tc.tile_pool(name="hst", bufs=3) as hpool,
            tc.tile_pool(name="probs", bufs=4) as probs_pool,
            tc.tile_pool(name="stage", bufs=4) as stage_pool,
            tc.tile_pool(name="pssm", bufs=2, space="PSUM") as ps_sm,
            tc.tile_pool(name="pssc", bufs=2, space="PSUM") as ps_sc,
        ):
            hsT_r = hsT.rearrange("p (t kt s) -> p t kt s", kt=KT, s=1024)

            # Tiny first DMA so the DMA queues spin up during the ~5us
            # engine preamble instead of delaying the first weight load.
            warm = singles.tile([128, 16], BF16, tag="warm")
            nc.sync.dma_start(out=warm, in_=w_ap["q"][:, 0:16])

            # DMA issue order tuned for startup: q/k weights first, then the
            # first hidden-state tile, then biases/wv/ones.
            w_sb = {}
            b_sb = {}
            for nm in ("q", "k"):
                wt = singles.tile([128, KT, 128], BF16, tag=f"w{nm}")
                nc.sync.dma_start(
                    out=wt, in_=w_ap[nm].rearrange("p (kt m) -> p kt m", m=128)
                )
                w_sb[nm] = wt

            hst_pre = {}

            def load_hst(b, n):
                t = b * NT + n
                hst = hpool.tile([128, KT, 1024], BF16, tag="hst")
                # one DMA per contraction slab so matmul k can start as soon
                # as slab k lands
                for k in range(KT):
                    nc.sync.dma_start(
                        out=hst[:, k : k + 1, :], in_=hsT_r[:, t, k : k + 1, :]
                    )
                return hst

            hst_pre[(0, 0)] = load_hst(0, 0)

            for nm in ("q", "k"):
                bt = singles.tile([128, 1], F32, tag=f"b{nm}")
                nc.sync.dma_start(out=bt, in_=b_ap[nm])
                b_sb[nm] = bt
            wt = singles.tile([128, KT, 128], BF16, tag="wv")
            nc.sync.dma_start(
                out=wt, in_=w_ap["v"].rearrange("p (kt m) -> p kt m", m=128)
            )
            w_sb["v"] = wt

            QT = big.tile([128, BS], BF16, tag="qt")
            KTt = big.tile([128, BS], BF16, tag="kt")
            # v store: per 128-key block, [seq 128, 65*2]: cols 0:64 head0 d,
            # 64 ones, 65:129 head1 d, 129 ones
            vfull = big.tile([128, B * NKB, 130], BF16, tag="vfull")
            nc.vector.memset(
                vfull.rearrange("p s (x o) -> p s x o", x=2)[:, :, :, 64:65], 1.0
            )

            def attn_scores(b, c):
                """Issue score matmuls + exp for chunk c; return PV context."""
                base = b * S
                lo = max(0, 2 * c - 2)
                hi = min(NKB, 2 * c + 4)
                n_kb = hi - lo
                q_sl = slice(base + c * W, base + (c + 1) * W)
                prs = {}
                for h in (0, 1):
                    d_sl = slice(h * 64, (h + 1) * 64)
                    sps = ps_sc.tile([128, 6, 256], F32, tag="sc", name=f"s{h}_{b}_{c}")
                    for i in range(n_kb):
                        kb = lo + i
                        k_sl = slice(base + kb * 128, base + (kb + 1) * 128)
                        nc.tensor.matmul(
                            sps[:, i, :],
                            lhsT=KTt[d_sl, k_sl],
                            rhs=QT[d_sl, q_sl],
                            start=True,
                            stop=True,
                        )
                    pr = probs_pool.tile(
                        [128, 6, 256], BF16, tag="probs", name=f"pr{h}_{b}_{c}"
                    )
                    nc.scalar.activation(
                        pr[:, 0:n_kb, :], sps[:, 0:n_kb, :], AFT.Exp
                    )
                    prs[h] = pr
                return (b, c, lo, n_kb, q_sl, prs)

            def attn_pv(ctx):
                b, c, lo, n_kb, q_sl, prs = ctx
                for h in (0, 1):
                    po = ps_sm.tile([65, 256], F32, tag="sm", name=f"pv{h}_{b}_{c}")
                    for i in range(n_kb):
                        slot = b * NKB + lo + i
                        nc.tensor.matmul(
                            po,
                            lhsT=vfull[:, slot, h * 65 : (h + 1) * 65],
                            rhs=prs[h][:, i, :],
                            start=(i == 0),
                            stop=(i == n_kb - 1),
                        )
                    st = stage_pool.tile([65, 256], F32, tag="stage")
                    nc.vector.tensor_copy(st, po)
                    nc.sync.dma_start(
                        out=outT[h * 65 : (h + 1) * 65, q_sl], in_=st
                    )

            pending = None
            for b in range(B):
                base = b * S
                c_done = 0
                for n in range(NT):
                    hst = hst_pre.pop((b, n), None)
                    if hst is None:
                        hst = load_hst(b, n)
                    # q/k projections: [feat, seq] layout, lhsT = weights
                    for nm, dest, scale, bias in (
                        ("q", QT, 1.0 / np.sqrt(D), b_sb["q"]),
                        ("k", KTt, 1.0, b_sb["k"]),
                    ):
                        for half in (0, 1):
                            hsl = slice(half * 512, (half + 1) * 512)
                            osl = slice(
                                base + n * 1024 + half * 512,
                                base + n * 1024 + (half + 1) * 512,
                            )
                            psp = ps_sm.tile(
                                [128, 512], F32, tag="sm", name=f"p{nm}_{b}_{n}_{half}"
                            )
                            for k in range(KT):
                                nc.tensor.matmul(
                                    psp,
                                    lhsT=w_sb[nm][:, k, :],
                                    rhs=hst[:, k, hsl],
                                    start=(k == 0),
                                    stop=(k == KT - 1),
                                )
                            nc.scalar.activation(
                                dest[:, osl], psp, AFT.Identity, bias=bias, scale=scale
                            )
                    # v projection: [seq, feat] layout, lhsT = hidden tile
                    for j in range(KT):
                        kb = n * 8 + j
                        ssl = slice(j * 128, (j + 1) * 128)
                        psv = ps_sm.tile(
                            [128, 128], F32, tag="sm", name=f"pv_{b}_{n}_{j}"
                        )
                        for k in range(KT):
                            nc.tensor.matmul(
                                psv,
                                lhsT=hst[:, k, ssl],
                                rhs=w_sb["v"][:, k, :],
                                start=(k == 0),
                                stop=(k == KT - 1),
                            )
                        slot = b * NKB + kb
                        nc.vector.tensor_copy(
                            vfull[:, slot, 0:130].rearrange("p (x o) -> p x o", x=2)[
                                :, :, 0:64
                            ],
                            psv.rearrange("p (x o) -> p x o", x=2),
                        )
                    # attention chunks whose window is now fully projected
                    c_hi = NCHUNK if n == NT - 1 else 4 * n + 3
                    for c in range(c_done, c_hi):
                        ctx = attn_scores(b, c)
                        if pending is not None:
                            attn_pv(pending)
                        pending = ctx
                    c_done = c_hi
            if pending is not None:
                attn_pv(pending)

    nc.compile()
    return nc


def get_nc():
    global _NC_CACHE
    if _NC_CACHE is None:
        _NC_CACHE = _build()
    return _NC_CACHE


def _w_prep(w):
    """[128, E] slice of W -> [128 partition, KT*128] with row p holding
    W^T[kt*128+p, :] for kt = 0..KT-1."""
    bf = ml_dtypes.bfloat16
    wt = w.T.reshape(KT, 128, 128)  # [kt, p, m]
    return np.ascontiguousarray(wt.transpose(1, 0, 2).reshape(128, KT * 128)).astype(bf)


def make_in_maps(hidden_states, Wq, bq, Wk, bk, Wv, bv):
    global _BV
    _BV = np.asarray(bv, np.float32)
    bf = ml_dtypes.bfloat16
    # [E, BS] -> [128(p), tile(8), kt(8), 1024] -> [128, 64K], row-contiguous
    hsTf = hidden_states.reshape(BS, E).T  # [E, BS]
    hsT = np.ascontiguousarray(
        hsTf.reshape(KT, 128, B * NT, 1024).transpose(1, 2, 0, 3).reshape(128, -1)
    ).astype(bf)
    in_maps = []
    for c in range(8):
        fsl = slice(c * 128, (c + 1) * 128)
        in_maps.append(
            {
                "hsT": hsT,
                "wqT": _w_prep(Wq[fsl]),
                "wkT": _w_prep(Wk[fsl]),
                "wvT": _w_prep(Wv[fsl]),
                "bq": np.ascontiguousarray(
                    (bq[fsl] / np.sqrt(D)).reshape(128, 1).astype(np.float32)
                ),
                "bk": np.ascontiguousarray(bk[fsl].reshape(128, 1).astype(np.float32)),
            }
        )
    return in_maps


def assemble(results):
    """results: list of 8 per-core dicts with 'outT' [130, BS] -> full [B,S,E]."""
    # boundary pad mass: chunk 0 row ii has ii unmasked zero-score pad keys,
    # chunk 15 row ii has 255-ii
    pad = np.zeros(S, np.float32)
    pad[:W] = np.arange(W, dtype=np.float32)
    pad[S - W :] = (W - 1) - np.arange(W, dtype=np.float32)
    pad_bs = np.tile(pad, B)  # [BS]

    out = np.empty((B, S, E), np.float32)
    for c in range(8):
        oT = np.asarray(results[c]["outT"], np.float32)  # [130, BS]
        for hl in (0, 1):
            num = oT[hl * 65 : hl * 65 + 64].T  # [BS, 64]
            den_dev = oT[hl * 65 + 64]  # [BS]
            den = den_dev + pad_bs
            fsl = slice(c * 128 + hl * 64, c * 128 + (hl + 1) * 64)
            full = (num + den_dev[:, None] * _BV[None, fsl]) / den[:, None]
            out.reshape(BS, E)[:, fsl] = full
    return out


def kernel(hidden_states, Wq, bq, Wk, bk, Wv, bv):
    hidden_states, Wq, bq, Wk, bk, Wv, bv = (
        np.asarray(x, np.float32) for x in (hidden_states, Wq, bq, Wk, bk, Wv, bv)
    )
    nc = get_nc()
    in_maps = make_in_maps(hidden_states, Wq, bq, Wk, bk, Wv, bv)
    res = run_bass_kernel_spmd(nc, in_maps, list(range(8)))
    return assemble(res.results)
